# revision 37
# baseline (speedup 1.0000x reference)
"""Segment mean-pool (BERT lattice embedding) Trainium2 Bass kernel.

Full-input contract: kernel(hidden[64,512,768] f32, word_ids[64,512] i32,
num_tokens=400) -> [64,400,768] f32.

Strategy: data-parallel over batch across 8 NeuronCores (8 samples each).
word_ids are NON-DECREASING per sample (HF tokenizer word_ids()), so with
r[b, s] = rank of piece s's word among the sample's DISTINCT words (host
prefix-scan of the 128 KB index tensor), the 128 pieces of chunk
j = [128j, 128j+128) span ranks [base_j, base_j + U) where U bounds the
max #distinct words per chunk (88 for the staged distribution, <= 128 for
ANY sorted input; checked at run time with a rebuild fallback). Per
(sample, chunk) the ragged segment MEAN is one scaled-one-hot matmul pair:

    A_j[p, u]   = (r[b,128j+p] - base_j == u) * recip[wid[b,128j+p]]
                  (one fused is_equal+mult tensor_scalar per chunk)
    psum_j[u,:] = A_j.T @ hidden[b, 128j:128j+128, :]   two banks: 512+256

Row p of A_j has one nonzero at its own rank offset, scaled by the word's
1/max(count,1) - psum holds final means and the post-matmul step is a pure
PSUM->SBUF bf16 cast, ONE [U,768] op per chunk (ACT for even j, DVE for odd
j - elementwise ops carry ~350 ns fixed cost, so fewer/bigger is faster).
Every hidden element enters the PE exactly once (3072 streamed cols per
sample) and everything runs in bf16 (~3e-3 rel err vs the 2e-2 gate). The
device emits per-chunk rank windows [B_LOC, U, J, H] bf16; the host maps
rank -> word id and scatter-adds windows into the final [400] rows
(consecutive windows overlap in at most the boundary word, and both
partials carry the same per-word 1/count scale, so plain addition is
exact).

DMA rings: bulk input prefetch on the sync HWDGE ring (whole shard fits
SBUF; one 6 KB-descriptor DMA per sample, sample 0 split per chunk to
start compute early); output half-samples split across the scalar and
gpsimd rings (both warmed at t=0 by the tiny index loads). The one-hot
build for sample b+1 is emitted ahead of sample b's DVE cast, and the
output tile pool is 6 deep, so neither the PE nor the casts ever stall
on DVE program order / output-DMA back-pressure (both measured stalls).
"""

import numpy as np
import ml_dtypes

B, S, H, T = 64, 512, 768, 400
N_CORES = 8
B_LOC = B // N_CORES  # samples per core
P = 128
J = S // P  # piece chunks per sample
N0 = 512  # psum bank split: matmul N=512 (bank 0) + N=256 (bank 1)
U_DEFAULT = 88  # max distinct words per 128-piece chunk (measured 88)

BF16 = ml_dtypes.bfloat16

_CACHED = {}


def build_program(u_width=U_DEFAULT):
    """Build + compile the single-core Bass program (same NEFF on all cores)."""
    import concourse.bass as bass  # noqa: F401
    import concourse.mybir as mybir
    import concourse.tile as tile
    from concourse import bacc

    assert u_width <= P

    nc = bacc.Bacc(
        "TRN2",
        target_bir_lowering=False,
        debug=False,
        enable_asserts=False,
        num_devices=N_CORES,
    )
    f32 = mybir.dt.float32
    bf16 = mybir.dt.bfloat16

    # hid_pjh[b, p, j, :] = hidden[b, 128j + p, :] in bf16 (host-packed so
    # every partition reads one contiguous J*H run per sample).
    hid_t = nc.dram_tensor("hid_pjh", [B_LOC, P, J, H], bf16, kind="ExternalInput").ap()
    # widl[p, b, j] = rank[b, 128j+p] - rank[b, 128j]  (bf16-exact, in [0, U))
    widl_t = nc.dram_tensor("wid_local", [P, B_LOC, J], bf16, kind="ExternalInput").ap()
    # iota3[p, j, u] = u (host-uploaded constant; avoids a slow gpsimd iota
    # on the critical path to the first one-hot build)
    iota_t = nc.dram_tensor("iota3", [P, J, u_width], bf16, kind="ExternalInput").ap()
    # recw[u, b, j] = 1/max(count of the word with rank base[b,j]+u, 1)
    recw_t = nc.dram_tensor(
        "recip_win", [u_width, B_LOC, J], f32, kind="ExternalInput"
    ).ap()
    # out[b, u, j, :] = rank-window row u of chunk j (rank base[b,j]+u)
    out_t = nc.dram_tensor(
        "out_loc", [B_LOC, u_width, J, H], bf16, kind="ExternalOutput"
    ).ap()

    with tile.TileContext(nc) as tc:
        with tc.tile_pool(name="const", bufs=1) as const_pool, \
             tc.tile_pool(name="hidp", bufs=B_LOC) as hid_pool, \
             tc.tile_pool(name="aTp", bufs=4) as aT_pool, \
             tc.tile_pool(name="outp", bufs=6) as out_pool, \
             tc.tile_pool(name="psum", bufs=4, space="PSUM") as psum_pool:

            # Index tensors FIRST: tiny and on the critical path to the
            # first one-hot build - they must sit ahead of the bulk input
            # samples in the scalar ring's FIFO. recw rides the gpsimd
            # ring (only needed by casts; also warms the output ring).
            widl_sb = const_pool.tile([P, B_LOC, J], bf16, name="widl_sb")
            nc.scalar.dma_start(out=widl_sb, in_=widl_t)
            iota3 = const_pool.tile([P, J, u_width], bf16, name="iota3")
            nc.scalar.dma_start(out=iota3, in_=iota_t)
            recw_sb = const_pool.tile([u_width, B_LOC, J], f32, name="recw_sb")
            nc.gpsimd.dma_start(out=recw_sb, in_=recw_t)

            # Bulk input prefetch, SPLIT ACROSS TWO HWDGE RINGS (sync +
            # scalar): the 16 DMA engines round-robin per queue, so two
            # input rings against one output ring give the input stream a
            # 2/3 fabric share - it must finish first, since the last
            # sample's compute+drain is the critical tail. Sample 0 split
            # per j-chunk so the first matmul starts as soon as chunk 0
            # lands. (The ACT engine issues its DMAs during the prologue,
            # before any casts need it.)
            hids = []
            for b in range(B_LOC):
                hid = hid_pool.tile([P, J, H], bf16, name=f"hid{b}", tag="hid")
                if b == 0:
                    for j in range(J):
                        nc.sync.dma_start(out=hid[:, j, :], in_=hid_t[b, :, j, :])
                elif b % 2 == 0:
                    nc.sync.dma_start(out=hid, in_=hid_t[b])
                else:
                    nc.scalar.dma_start(out=hid, in_=hid_t[b])
                hids.append(hid)

            def build_onehot(b):
                # All 4 chunk one-hots in ONE DVE op: compare iota3 against
                # widl broadcast over u (stride-0 free dim).
                aT = aT_pool.tile([P, J, u_width], bf16, name="aT", tag="aT")
                widl_b = widl_sb[:, b, :].unsqueeze(2).broadcast_to([P, J, u_width])
                nc.vector.tensor_tensor(aT, iota3, widl_b, op=mybir.AluOpType.is_equal)
                return aT

            aTs = {0: build_onehot(0)}
            for b in range(B_LOC):
                hid = hids[b]
                # One-sample lookahead: emit the NEXT sample's one-hot build
                # ahead of this sample's DVE casts, so the PE never waits on
                # an aT stuck behind casts in DVE program order.
                if b + 1 < B_LOC:
                    aTs[b + 1] = build_onehot(b + 1)
                aT = aTs.pop(b)
                om = out_pool.tile([u_width, J, H], bf16, name="om", tag="om")
                for j in range(J):
                    # One [U, 768] psum tile spanning two banks; each matmul
                    # write stays inside one bank (512 | 256).
                    ps = psum_pool.tile([u_width, H], f32, name="ps", tag="ps")
                    lhsT = aT[:, j, :]
                    nc.tensor.matmul(ps[:, 0:N0], lhsT, hid[:, j, 0:N0], start=True, stop=True)
                    nc.tensor.matmul(ps[:, N0:H], lhsT, hid[:, j, N0:H], start=True, stop=True)
                    # Whole-chunk PSUM -> SBUF bf16 cast, scaled by the
                    # window recips (per-partition scalar); ACT and DVE
                    # alternate so each carries two per sample.
                    rec = recw_sb[:, b, j : j + 1]
                    if j % 2 == 0:
                        nc.scalar.mul(om[:, j, :], ps, rec)
                    else:
                        nc.vector.tensor_scalar_mul(om[:, j, :], ps, rec)
                # Output halves on two rings (a single HWDGE queue caps at
                # ~150 GB/s for writes): the first half streams live on the
                # gpsimd ring; the second half rides the SCALAR ring, whose
                # FIFO places it behind that ring's input samples - it
                # drains in bulk on a second queue exactly when the input
                # finishes and fabric share frees up.
                nc.gpsimd.dma_start(out=out_t[b, :, 0:2, :], in_=om[:, 0:2, :])
                nc.scalar.dma_start(out=out_t[b, :, 2:4, :], in_=om[:, 2:4, :])

    nc.compile()
    return nc


def _ranks(wid):
    """rank[b, s] = index of piece s's word among the sample's distinct words."""
    rank = np.zeros_like(wid)
    rank[:, 1:] = np.cumsum(np.diff(wid, axis=1) != 0, axis=1)
    return rank


def _pack_inputs(hidden, word_ids, u_width):
    """Full-batch host prep: bf16 cast + per-core input maps."""
    hidden = np.ascontiguousarray(np.asarray(hidden), dtype=np.float32).reshape(B, S, H)
    wid = np.ascontiguousarray(np.asarray(word_ids), dtype=np.int32).reshape(B, S)

    hid16 = hidden.astype(BF16)

    counts = np.zeros((B, T), np.int64)
    np.add.at(counts, (np.repeat(np.arange(B), S), wid.reshape(-1)), 1)
    recip = (1.0 / np.maximum(counts, 1)).astype(np.float32)  # [B, T]

    rank = _ranks(wid)
    base = rank[:, ::P]  # [B, J] first rank of each chunk

    # recip in rank space: rr[b, r] = recip of the r-th distinct word
    rr = np.ones((B, S + P), np.float32)
    first = np.ones((B, S), bool)
    first[:, 1:] = wid[:, 1:] != wid[:, :-1]
    bidx, sidx = np.nonzero(first)
    rr[bidx, rank[bidx, sidx]] = recip[bidx, wid[bidx, sidx]]
    # recw[b, j, u] = rr[b, base[b, j] + u]
    idx = base[:, :, None] + np.arange(u_width)[None, None, :]
    recw = rr[np.arange(B)[:, None, None], idx]  # [B, J, U]

    in_maps = []
    for i in range(N_CORES):
        sl = slice(i * B_LOC, (i + 1) * B_LOC)
        hs = np.ascontiguousarray(
            hid16[sl].reshape(B_LOC, J, P, H).transpose(0, 2, 1, 3)
        )
        wl = np.ascontiguousarray(
            (rank[sl].reshape(B_LOC, J, P) - base[sl][:, :, None])
            .transpose(2, 0, 1)
            .astype(BF16)
        )
        rw = np.ascontiguousarray(recw[sl].transpose(2, 0, 1).astype(np.float32))
        io3 = np.ascontiguousarray(
            np.broadcast_to(
                np.arange(u_width, dtype=np.float32), (P, J, u_width)
            ).astype(BF16)
        )
        in_maps.append(
            {"hid_pjh": hs, "wid_local": wl, "recip_win": rw, "iota3": io3}
        )
    return in_maps


def _combine(core_outs, word_ids, u_width):
    """Scatter-add per-chunk rank windows into the full [B, T, H] output."""
    wid = np.asarray(word_ids, np.int32).reshape(B, S)
    rank = _ranks(wid)
    base = rank[:, ::P]  # [B, J]
    ndist = rank[:, -1] + 1  # distinct words per sample
    out = np.zeros((B, T, H), np.float32)
    for i, arr in enumerate(core_outs):
        a = np.asarray(arr).astype(np.float32)  # [B_LOC, U, J, H]
        for b in range(B_LOC):
            gb = i * B_LOC + b
            # uniq[r] = word id of rank r (first occurrence per run)
            first = np.ones(S, bool)
            first[1:] = wid[gb, 1:] != wid[gb, :-1]
            uniq = wid[gb, first]  # [ndist]
            for j in range(J):
                r0 = int(base[gb, j])
                w = min(u_width, int(ndist[gb]) - r0)
                # ranks within one window are unique -> fancy += is safe
                out[gb, uniq[r0 : r0 + w]] += a[b, :w, j]
    return out


def _u_required(word_ids):
    wid = np.asarray(word_ids, np.int32).reshape(B, S)
    rank = _ranks(wid)
    wmax = 0
    for j in range(J):
        wmax = max(wmax, int((rank[:, (j + 1) * P - 1] - rank[:, j * P]).max()) + 1)
    return wmax


def run(hidden, word_ids, trace=False, **trace_kwargs):
    from concourse import bass_utils

    u_width = max(U_DEFAULT, _u_required(word_ids))
    if u_width not in _CACHED:
        _CACHED[u_width] = build_program(u_width)
    nc = _CACHED[u_width]
    in_maps = _pack_inputs(hidden, word_ids, u_width)
    res = bass_utils.run_bass_kernel_spmd(
        nc, in_maps, core_ids=list(range(N_CORES)), trace=trace, **trace_kwargs
    )
    out = _combine(
        [res.results[i]["out_loc"] for i in range(N_CORES)], word_ids, u_width
    )
    return out, res


def kernel(hidden, word_ids, num_tokens=None, **_unused):
    out, _ = run(hidden, word_ids, trace=False)
    return out


# revision 39
# speedup vs baseline: 1.1426x; 1.1426x over previous
"""Segment mean-pool (BERT lattice embedding) Trainium2 Bass kernel.

Full-input contract: kernel(hidden[64,512,768] f32, word_ids[64,512] i32,
num_tokens=400) -> [64,400,768] f32.

Strategy: data-parallel over batch across 8 NeuronCores (8 samples each).
word_ids are NON-DECREASING per sample (HF tokenizer word_ids()), so with
r[b, s] = rank of piece s's word among the sample's DISTINCT words (host
prefix-scan of the 128 KB index tensor), the 128 pieces of chunk
j = [128j, 128j+128) span ranks [base_j, base_j + U) where U bounds the
max #distinct words per chunk (88 for the staged distribution, <= 128 for
ANY sorted input; checked at run time with a rebuild fallback). Per
(sample, chunk) the ragged segment MEAN is one scaled-one-hot matmul pair:

    A_j[p, u]   = (r[b,128j+p] - base_j == u) * recip[wid[b,128j+p]]
                  (one fused is_equal+mult tensor_scalar per chunk)
    psum_j[u,:] = A_j.T @ hidden[b, 128j:128j+128, :]   two banks: 512+256

Row p of A_j has one nonzero at its own rank offset, scaled by the word's
1/max(count,1) - psum holds final means and the post-matmul step is a pure
PSUM->SBUF bf16 cast, ONE [U,768] op per chunk (ACT for even j, DVE for odd
j - elementwise ops carry ~350 ns fixed cost, so fewer/bigger is faster).
Every hidden element enters the PE exactly once (3072 streamed cols per
sample) and everything runs in bf16 (~3e-3 rel err vs the 2e-2 gate). The
device emits per-chunk rank windows [B_LOC, U, J, H] bf16; the host maps
rank -> word id and scatter-adds windows into the final [400] rows
(consecutive windows overlap in at most the boundary word, and both
partials carry the same per-word 1/count scale, so plain addition is
exact).

DMA rings: bulk input prefetch on the sync HWDGE ring (whole shard fits
SBUF; one 6 KB-descriptor DMA per sample, sample 0 split per chunk to
start compute early); output half-samples split across the scalar and
gpsimd rings (both warmed at t=0 by the tiny index loads). The one-hot
build for sample b+1 is emitted ahead of sample b's DVE cast, and the
output tile pool is 6 deep, so neither the PE nor the casts ever stall
on DVE program order / output-DMA back-pressure (both measured stalls).
"""

import numpy as np
import ml_dtypes

B, S, H, T = 64, 512, 768, 400
N_CORES = 8
B_LOC = B // N_CORES  # samples per core
P = 128
J = S // P  # piece chunks per sample
N0 = 512  # psum bank split: matmul N=512 (bank 0) + N=256 (bank 1)
U_DEFAULT = 88  # max distinct words per 128-piece chunk (measured 88)

BF16 = ml_dtypes.bfloat16

_CACHED = {}


def build_program(u_width=U_DEFAULT):
    """Build + compile the single-core Bass program (same NEFF on all cores)."""
    import concourse.bass as bass  # noqa: F401
    import concourse.mybir as mybir
    import concourse.tile as tile
    from concourse import bacc

    assert u_width <= P

    nc = bacc.Bacc(
        "TRN2",
        target_bir_lowering=False,
        debug=False,
        enable_asserts=False,
        num_devices=N_CORES,
    )
    f32 = mybir.dt.float32
    bf16 = mybir.dt.bfloat16

    # hid_pjh[b, p, j, :] = hidden[b, 128j + p, :] in bf16 (host-packed so
    # every partition reads one contiguous J*H run per sample).
    hid_t = nc.dram_tensor("hid_pjh", [B_LOC, P, J, H], bf16, kind="ExternalInput").ap()
    # widl[p, b, j] = rank[b, 128j+p] - rank[b, 128j]  (bf16-exact, in [0, U))
    widl_t = nc.dram_tensor("wid_local", [P, B_LOC, J], bf16, kind="ExternalInput").ap()
    # iota3[p, j, u] = u (host-uploaded constant; avoids a slow gpsimd iota
    # on the critical path to the first one-hot build)
    iota_t = nc.dram_tensor("iota3", [P, J, u_width], bf16, kind="ExternalInput").ap()
    # recw[u, b, j] = 1/max(count of the word with rank base[b,j]+u, 1)
    recw_t = nc.dram_tensor(
        "recip_win", [u_width, B_LOC, J], f32, kind="ExternalInput"
    ).ap()
    # out[b, u, j, :] = rank-window row u of chunk j (rank base[b,j]+u)
    out_t = nc.dram_tensor(
        "out_loc", [B_LOC, u_width, J, H], bf16, kind="ExternalOutput"
    ).ap()

    with tile.TileContext(nc) as tc:
        with tc.tile_pool(name="const", bufs=1) as const_pool, \
             tc.tile_pool(name="hidp", bufs=B_LOC) as hid_pool, \
             tc.tile_pool(name="aTp", bufs=4) as aT_pool, \
             tc.tile_pool(name="outp", bufs=6) as out_pool, \
             tc.tile_pool(name="psum", bufs=4, space="PSUM") as psum_pool:

            # Index tensors FIRST: tiny and on the critical path to the
            # first one-hot build - they must sit ahead of the bulk input
            # samples in the scalar ring's FIFO. recw rides the gpsimd
            # ring (only needed by casts; also warms the output ring).
            widl_sb = const_pool.tile([P, B_LOC, J], bf16, name="widl_sb")
            nc.scalar.dma_start(out=widl_sb, in_=widl_t)
            iota3 = const_pool.tile([P, J, u_width], bf16, name="iota3")
            nc.scalar.dma_start(out=iota3, in_=iota_t)
            recw_sb = const_pool.tile([u_width, B_LOC, J], f32, name="recw_sb")
            nc.gpsimd.dma_start(out=recw_sb, in_=recw_t)

            # Bulk input prefetch on the sync HWDGE ring: streams the whole
            # shard back-to-back from t=0. Sample 0 split per j-chunk so
            # the first matmul starts as soon as chunk 0 lands. (Splitting
            # input across two rings finishes it ~5 us earlier, but the
            # output then bottlenecks on a single ~150 GB/s write queue -
            # measured net-worse.)
            hids = []
            for b in range(B_LOC):
                hid = hid_pool.tile([P, J, H], bf16, name=f"hid{b}", tag="hid")
                if b == 0:
                    for j in range(J):
                        nc.sync.dma_start(out=hid[:, j, :], in_=hid_t[b, :, j, :])
                else:
                    nc.sync.dma_start(out=hid, in_=hid_t[b])
                hids.append(hid)

            def build_onehot(b):
                # All 4 chunk one-hots in ONE DVE op: compare iota3 against
                # widl broadcast over u (stride-0 free dim).
                aT = aT_pool.tile([P, J, u_width], bf16, name="aT", tag="aT")
                widl_b = widl_sb[:, b, :].unsqueeze(2).broadcast_to([P, J, u_width])
                nc.vector.tensor_tensor(aT, iota3, widl_b, op=mybir.AluOpType.is_equal)
                return aT

            aTs = {0: build_onehot(0)}
            for b in range(B_LOC):
                hid = hids[b]
                # One-sample lookahead: emit the NEXT sample's one-hot build
                # ahead of this sample's DVE casts, so the PE never waits on
                # an aT stuck behind casts in DVE program order.
                if b + 1 < B_LOC:
                    aTs[b + 1] = build_onehot(b + 1)
                aT = aTs.pop(b)
                om = out_pool.tile([u_width, J, H], bf16, name="om", tag="om")
                for j in range(J):
                    # One [U, 768] psum tile spanning two banks; each matmul
                    # write stays inside one bank (512 | 256).
                    ps = psum_pool.tile([u_width, H], f32, name="ps", tag="ps")
                    lhsT = aT[:, j, :]
                    nc.tensor.matmul(ps[:, 0:N0], lhsT, hid[:, j, 0:N0], start=True, stop=True)
                    nc.tensor.matmul(ps[:, N0:H], lhsT, hid[:, j, N0:H], start=True, stop=True)
                    # Whole-chunk PSUM -> SBUF bf16 cast, scaled by the
                    # window recips (per-partition scalar); ACT and DVE
                    # alternate so each carries two per sample.
                    rec = recw_sb[:, b, j : j + 1]
                    if j % 2 == 0:
                        nc.scalar.mul(om[:, j, :], ps, rec)
                    else:
                        nc.vector.tensor_scalar_mul(om[:, j, :], ps, rec)
                # Output halves split across the scalar and gpsimd rings
                # (a single HWDGE queue caps at ~150 GB/s for writes; two
                # queues drain the windows at ~2x that). Issued after the
                # full cast loop so neither cast engine stalls waiting on
                # the other's half.
                nc.scalar.dma_start(out=out_t[b, :, 0:2, :], in_=om[:, 0:2, :])
                nc.gpsimd.dma_start(out=out_t[b, :, 2:4, :], in_=om[:, 2:4, :])

    nc.compile()
    return nc


def _ranks(wid):
    """rank[b, s] = index of piece s's word among the sample's distinct words."""
    rank = np.zeros_like(wid)
    rank[:, 1:] = np.cumsum(np.diff(wid, axis=1) != 0, axis=1)
    return rank


def _pack_inputs(hidden, word_ids, u_width):
    """Full-batch host prep: bf16 cast + per-core input maps."""
    hidden = np.ascontiguousarray(np.asarray(hidden), dtype=np.float32).reshape(B, S, H)
    wid = np.ascontiguousarray(np.asarray(word_ids), dtype=np.int32).reshape(B, S)

    hid16 = hidden.astype(BF16)

    counts = np.zeros((B, T), np.int64)
    np.add.at(counts, (np.repeat(np.arange(B), S), wid.reshape(-1)), 1)
    recip = (1.0 / np.maximum(counts, 1)).astype(np.float32)  # [B, T]

    rank = _ranks(wid)
    base = rank[:, ::P]  # [B, J] first rank of each chunk

    # recip in rank space: rr[b, r] = recip of the r-th distinct word
    rr = np.ones((B, S + P), np.float32)
    first = np.ones((B, S), bool)
    first[:, 1:] = wid[:, 1:] != wid[:, :-1]
    bidx, sidx = np.nonzero(first)
    rr[bidx, rank[bidx, sidx]] = recip[bidx, wid[bidx, sidx]]
    # recw[b, j, u] = rr[b, base[b, j] + u]
    idx = base[:, :, None] + np.arange(u_width)[None, None, :]
    recw = rr[np.arange(B)[:, None, None], idx]  # [B, J, U]

    in_maps = []
    for i in range(N_CORES):
        sl = slice(i * B_LOC, (i + 1) * B_LOC)
        hs = np.ascontiguousarray(
            hid16[sl].reshape(B_LOC, J, P, H).transpose(0, 2, 1, 3)
        )
        wl = np.ascontiguousarray(
            (rank[sl].reshape(B_LOC, J, P) - base[sl][:, :, None])
            .transpose(2, 0, 1)
            .astype(BF16)
        )
        rw = np.ascontiguousarray(recw[sl].transpose(2, 0, 1).astype(np.float32))
        io3 = np.ascontiguousarray(
            np.broadcast_to(
                np.arange(u_width, dtype=np.float32), (P, J, u_width)
            ).astype(BF16)
        )
        in_maps.append(
            {"hid_pjh": hs, "wid_local": wl, "recip_win": rw, "iota3": io3}
        )
    return in_maps


def _combine(core_outs, word_ids, u_width):
    """Scatter-add per-chunk rank windows into the full [B, T, H] output."""
    wid = np.asarray(word_ids, np.int32).reshape(B, S)
    rank = _ranks(wid)
    base = rank[:, ::P]  # [B, J]
    ndist = rank[:, -1] + 1  # distinct words per sample
    out = np.zeros((B, T, H), np.float32)
    for i, arr in enumerate(core_outs):
        a = np.asarray(arr).astype(np.float32)  # [B_LOC, U, J, H]
        for b in range(B_LOC):
            gb = i * B_LOC + b
            # uniq[r] = word id of rank r (first occurrence per run)
            first = np.ones(S, bool)
            first[1:] = wid[gb, 1:] != wid[gb, :-1]
            uniq = wid[gb, first]  # [ndist]
            for j in range(J):
                r0 = int(base[gb, j])
                w = min(u_width, int(ndist[gb]) - r0)
                # ranks within one window are unique -> fancy += is safe
                out[gb, uniq[r0 : r0 + w]] += a[b, :w, j]
    return out


def _u_required(word_ids):
    wid = np.asarray(word_ids, np.int32).reshape(B, S)
    rank = _ranks(wid)
    wmax = 0
    for j in range(J):
        wmax = max(wmax, int((rank[:, (j + 1) * P - 1] - rank[:, j * P]).max()) + 1)
    return wmax


def run(hidden, word_ids, trace=False, **trace_kwargs):
    from concourse import bass_utils

    u_width = max(U_DEFAULT, _u_required(word_ids))
    if u_width not in _CACHED:
        _CACHED[u_width] = build_program(u_width)
    nc = _CACHED[u_width]
    in_maps = _pack_inputs(hidden, word_ids, u_width)
    res = bass_utils.run_bass_kernel_spmd(
        nc, in_maps, core_ids=list(range(N_CORES)), trace=trace, **trace_kwargs
    )
    out = _combine(
        [res.results[i]["out_loc"] for i in range(N_CORES)], word_ids, u_width
    )
    return out, res


def kernel(hidden, word_ids, num_tokens=None, **_unused):
    out, _ = run(hidden, word_ids, trace=False)
    return out
